# revision 1
# baseline (speedup 1.0000x reference)
"""Trainium2 Bass kernel for the 2-layer linear-GRU decoder (nn_Decoder_WOG).

Self-contained: hardcodes shapes B=64, T=12, N=2048, Din=2, HD=64, OUT=1,
8 NeuronCores data-parallel over the B*N token dimension.

Layout strategy (per core, 16384 tokens):
  - feature-major: features on SBUF partitions, tokens on the free dim
  - F=512-token chunks; flat software pipeline over (step t, chunk c)
  - layer-1 processing lags layer-0 by exactly one step (NCH slots), so both
    layers' states pair in the SAME state column: S[:, c] = [s0(c); s1(c)],
    where s0 is at step t and s1 at step t-1
  - every matmul is a K=128 full-row matmul with zero-padded weights,
    merged across layers (one Z matmul computes both z0 and z1); this
    avoids cross-row-tile PSUM accumulation, which faults on TRN2
  - elementwise cell update as 4 paired [128, F] DVE scalar_tensor_tensor
    ops with per-partition biases fused; Z/H PSUM banks evacuated to bf16
    SBUF by the scalar engine (ACT) to keep the DVE in 2x 16-bit mode
  - out-projections pair two iterations into one PSUM bank (weight in
    column 0 for even slots, column 64 for odd) -> one ACT evac + DMA
    per two iterations

Cell algebra per layer (no nonlinearities in this model):
  zp = s@Bz + x@Az ; rp likewise ; rs = (rp + br) * s
  hp = rs@Bh + x@Ah ; M = (hp + bh) - s ; P2 = (zp + bz) * M
  s' = (hp + bh) - P2   ==  z*s + (1-z)*h  with z = zp+bz, h = hp+bh
"""

import numpy as np
import ml_dtypes

import concourse.bass as bass
import concourse.tile as tile
from concourse import bacc, mybir
from concourse.bass_utils import run_bass_kernel_spmd

BF16 = np.float16
F = 512
HD = 64
NCORE = 8
OS = 2    # out-projection emission shift (iterations)


def _build(T, NCH, reps=1):
    NITER = T * NCH
    TOK = NCH * F
    dt = mybir.dt
    AO = mybir.AluOpType
    NSLAB = 6 + 3 * T
    WCOLS = NSLAB * 128
    CC = WCOLS + TOK  # constants blob columns: weight slabs then x columns

    nc = bacc.Bacc("TRN2", target_bir_lowering=False, debug=False,
                   num_devices=NCORE)
    cst_d = nc.dram_tensor("cst", [128, CC], dt.float16, kind="ExternalInput")
    sp_d = nc.dram_tensor("sp", [128, TOK], dt.float16, kind="ExternalInput")
    bp_d = nc.dram_tensor("bp", [128, 3], dt.float32, kind="ExternalInput")
    out_d = nc.dram_tensor("out", [T, TOK], dt.float32, kind="ExternalOutput")

    with tile.TileContext(nc) as tc:
        with (
            tc.tile_pool(name="const", bufs=1) as cpool,
            tc.tile_pool(name="state", bufs=1) as spool,
            tc.tile_pool(name="work", bufs=4) as wpool,
            tc.tile_pool(name="ostage", bufs=3) as opool,
            tc.tile_pool(name="psum", bufs=2, space="PSUM") as ppool,
        ):
            csb = cpool.tile([128, CC], dt.float16)
            bsb = cpool.tile([128, 3], dt.float32)
            S_all = spool.tile([128, TOK], dt.float16)
            # Load state columns individually so the first pipeline bodies
            # can start as soon as their own chunk has landed (bacc's
            # sync-wait legalization handles the fan-in; no barrier needed).
            nc.gpsimd.dma_start(out=csb[:], in_=cst_d[:])
            nc.gpsimd.dma_start(out=bsb[:], in_=bp_d[:])
            nc.gpsimd.dma_start(out=S_all[:], in_=sp_d[:])

            def slab(idx):
                return csb[:, idx * 128:(idx + 1) * 128]

            WZ, WR, WH, WHT, WOE, WOO = (slab(i) for i in range(6))
            bz, br, bh = (bsb[:, i:i + 1] for i in range(3))
            out_view = out_d[:].rearrange("t (c f) -> (t c) f", f=F)

            def Sfc(c):
                return S_all[:, c * F:(c + 1) * F]

            mm = nc.tensor.matmul
            stt = nc.vector.scalar_tensor_tensor

            # Software pipeline: slot j's phases are emitted across bodies so
            # no engine's in-order queue ever waits on same-body work:
            #   body j:   Z/R matmuls + RS op (slot j)
            #   body j+1: H matmuls + Z/H evacs (slot j)
            #   body j+2: M/P2/S' update cluster (slot j)
            #   body j+3: out-projection (slot j)
            def active(j):
                return (j < NITER, NCH <= j < NITER + NCH)

            for rep in range(reps):
              zr_ph = {}
              up_ph = {}
              ps_o_box = [None]
              for i in range(NITER + NCH + 4):
                  l0, l1 = active(i)
                  if l0 or l1:
                      c = i % NCH
                      t = i // NCH
                      ps_z = ppool.tile([128, F], dt.float32, tag="Z")
                      ps_r = ppool.tile([128, F], dt.float32, tag="R")
                      S = Sfc(c)
                      if l0:
                          xap = csb[:, WCOLS + c * F:WCOLS + (c + 1) * F]
                          XZ, XR = (slab(6 + 3 * t + k) for k in range(2))
                      mm(ps_z[:], WZ, S, start=True, stop=not l0,
                         tile_position=(0, 0))
                      if l0:
                          mm(ps_z[:], XZ, xap, start=False, stop=True,
                             tile_position=(0, 0))
                      mm(ps_r[:], WR, S, start=True, stop=not l0,
                         tile_position=(0, 0))
                      if l0:
                          mm(ps_r[:], XR, xap, start=False, stop=True,
                             tile_position=(0, 0))
                      zr_ph[i] = (l0, l1, c, t, ps_z, ps_r)

                  # state-update cluster for slot i-2 (before RS so the DVE
                  # queue always has ready work first)
                  j = i - 2
                  if j in up_ph:
                      l0j, l1j, cj, Ztj, Htj = up_ph.pop(j)
                      Sj = Sfc(cj)
                      slj = (slice(0, 128) if (l0j and l1j)
                             else (slice(0, 64) if l0j else slice(64, 128)))
                      Mt = wpool.tile([128, F], dt.float16, tag="Mt")
                      P2 = wpool.tile([128, F], dt.float16, tag="P2")
                      stt(out=Mt[:], in0=Htj[:], scalar=bh, in1=Sj,
                          op0=AO.add, op1=AO.subtract)
                      stt(out=P2[:], in0=Ztj[:], scalar=bz, in1=Mt[:],
                          op0=AO.add, op1=AO.mult)
                      stt(out=S_all[slj, cj * F:(cj + 1) * F],
                          in0=Htj[slj, :], scalar=bh[slj, :], in1=P2[slj, :],
                          op0=AO.add, op1=AO.subtract)

                  # RS op for slot i
                  if l0 or l1:
                      RS = wpool.tile([128, F], dt.float16, tag="RS")
                      stt(out=RS[:], in0=ps_r[:], scalar=br, in1=S,
                          op0=AO.add, op1=AO.mult)
                      zr_ph[i] = zr_ph[i] + (RS,)

                  # H matmuls + evacs for slot i-1
                  j = i - 1
                  if j in zr_ph:
                      l0j, l1j, cj, tj, ps_zj, ps_rj, RSj = zr_ph.pop(j)
                      Sj = Sfc(cj)
                      ps_h = ppool.tile([128, F], dt.float32, tag="H")
                      mm(ps_h[:], WH, RSj[:], start=True, stop=False,
                         tile_position=(0, 0))
                      if l0j:
                          XH = slab(6 + 3 * tj + 2)
                          xapj = csb[:, WCOLS + cj * F:WCOLS + (cj + 1) * F]
                          mm(ps_h[:], XH, xapj, start=False, stop=False,
                             tile_position=(0, 0))
                      mm(ps_h[:], WHT, Sj, start=False, stop=True,
                         tile_position=(0, 0))
                      Zt = wpool.tile([128, F], dt.float16, tag="Zt")
                      Ht = wpool.tile([128, F], dt.float16, tag="Ht")
                      nc.scalar.copy(out=Zt[:], in_=ps_zj[:])
                      nc.scalar.copy(out=Ht[:], in_=ps_h[:])
                      up_ph[j] = (l0j, l1j, cj, Zt, Ht)

                  # out-projection for slot oi = i - 3 (reads updated s1')
                  oi = i - 3
                  if NCH <= oi < NITER + NCH:
                      cs_out = oi - NCH
                      So = Sfc(oi % NCH)
                      if cs_out % 2 == 0:
                          ps_o = ppool.tile([128, F], dt.float32, tag="O")
                          ps_o_box[0] = ps_o
                          mm(ps_o[:], WOE, So, start=True, stop=False,
                             tile_position=(0, 0))
                      else:
                          ps_o = ps_o_box[0]
                          mm(ps_o[:], WOO, So, start=False, stop=True,
                             tile_position=(0, 0))
                          ost = opool.tile([128, F], dt.float32, tag="ost")
                          nc.scalar.copy(out=ost[:], in_=ps_o[:])
                          nc.gpsimd.dma_start(out=out_view[cs_out - 1:cs_out, :],
                                              in_=ost[0:1, :])
                          nc.gpsimd.dma_start(out=out_view[cs_out:cs_out + 1, :],
                                              in_=ost[64:65, :])
    nc.compile()
    return nc


def _pack_weights(T, Wzr0, Wh0, Wzr1, Wh1, W_out):
    NSLAB = 6 + 3 * T
    wp = np.zeros((128, NSLAB * 128), np.float32)

    def sl(i):
        return wp[:, i * 128:(i + 1) * 128]

    # WZ: cols 0:64 -> z0 state part [Bz0; 0], cols 64:128 -> z1 = Wzr1 z-cols
    sl(0)[0:64, 0:64] = Wzr0[2:66, 0:HD]
    sl(0)[:, 64:128] = Wzr1[:, 0:HD]
    # WR
    sl(1)[0:64, 0:64] = Wzr0[2:66, HD:]
    sl(1)[:, 64:128] = Wzr1[:, HD:]
    # WH (rhs = RS = [rs0; rs1]): cols 0:64 -> [Bh0; 0], cols 64:128 -> [0; Wh1b]
    sl(2)[0:64, 0:64] = Wh0[2:66, :]
    sl(2)[64:128, 64:128] = Wh1[HD:, :]
    # WHT (rhs = S = [s0'; s1]): cols 64:128 -> [Wh1t; 0]
    sl(3)[0:64, 64:128] = Wh1[0:HD, :]
    # WOE / WOO: out projection, even slot in col 0, odd slot in col 64
    sl(4)[64:128, 0:1] = W_out
    sl(5)[64:128, 64:65] = W_out
    # x-projection slabs per step: rhs = x-region (x at rows 64+2t+d)
    for tt in range(T):
        for k, W in enumerate((Wzr0[0:2, 0:HD], Wzr0[0:2, HD:], Wh0[0:2, :])):
            s = sl(6 + 3 * tt + k)
            s[64 + 2 * tt:64 + 2 * tt + 2, 0:64] = W
    return wp


def _pack_biases(bzr0, bh0, bzr1, bh1):
    bp = np.zeros((128, 3), np.float32)
    bp[:, 0] = np.concatenate([bzr0[:HD], bzr1[:HD]])   # bz
    bp[:, 1] = np.concatenate([bzr0[HD:], bzr1[HD:]])   # br
    bp[:, 2] = np.concatenate([bh0, bh1])               # bh
    return bp


_NC_CACHE = {}


def _get_nc(T, NCH, reps=1):
    key = (T, NCH, reps)
    if key not in _NC_CACHE:
        _NC_CACHE[key] = _build(T, NCH, reps)
    return _NC_CACHE[key]


def run_cores(x_cores, s0_cores, s1_cores, T, NCH,
              Wzr0, bzr0, Wh0, bh0, Wzr1, bzr1, Wh1, bh1, W_out,
              **run_kwargs):
    """x_cores: list of [T, 2, TOK] fp32; s*_cores: list of [64, TOK] fp32.
    Returns (list of [T, TOK] fp32 outputs (without b_out), BassKernelResults)."""
    TOK = NCH * F
    nc = _get_nc(T, NCH, run_kwargs.pop("reps", 1))
    wp = _pack_weights(T, Wzr0, Wh0, Wzr1, Wh1, W_out)
    bp = _pack_biases(bzr0, bh0, bzr1, bh1)
    in_maps = []
    for x_core, s0, s1 in zip(x_cores, s0_cores, s1_cores):
        xpart = np.zeros((128, TOK), np.float32)
        xpart[64:64 + 2 * T] = x_core.reshape(2 * T, TOK)
        cst = np.concatenate([wp, xpart], axis=1).astype(BF16)
        sp = np.concatenate([s0, s1], 0).astype(BF16)
        in_maps.append({"cst": cst, "sp": sp, "bp": bp})
    res = run_bass_kernel_spmd(nc, in_maps, core_ids=list(range(len(in_maps))),
                               **run_kwargs)
    return [r["out"] for r in res.results], res


def kernel(x, init_state, Wzr0, bzr0, Wh0, bh0, Wzr1, bzr1, Wh1, bh1,
           W_out, b_out):
    x = np.asarray(x)
    init_state = np.asarray(init_state)
    args = [np.asarray(a) for a in
            (Wzr0, bzr0, Wh0, bh0, Wzr1, bzr1, Wh1, bh1, W_out)]
    b_out = np.asarray(b_out)

    B, T, N, Din = x.shape
    assert (B, T, N, Din) == (64, 12, 2048, 2), x.shape
    BPC = B // NCORE
    TOK = BPC * N
    NCH = TOK // F

    x_cores, s0_cores, s1_cores = [], [], []
    for core in range(NCORE):
        bs = slice(core * BPC, (core + 1) * BPC)
        x_cores.append(x[bs].transpose(1, 3, 0, 2).reshape(T, Din, TOK))
        s0_cores.append(init_state[0, bs].transpose(2, 0, 1).reshape(HD, TOK))
        s1_cores.append(init_state[1, bs].transpose(2, 0, 1).reshape(HD, TOK))

    outs, _ = run_cores(x_cores, s0_cores, s1_cores, T, NCH, *args)
    out = np.stack(outs).reshape(NCORE, T, BPC, N).transpose(0, 2, 1, 3)
    out = out.reshape(B, T, N, 1) + b_out[0]
    return out.astype(np.float32)



# revision 6
# speedup vs baseline: 773.4653x; 773.4653x over previous
"""Trainium2 Bass kernel for the 2-layer linear-GRU decoder (nn_Decoder_WOG).

Self-contained: hardcodes shapes B=64, T=12, N=2048, Din=2, HD=64, OUT=1,
8 NeuronCores data-parallel over the B*N token dimension.

Layout strategy (per core, 16384 tokens):
  - feature-major: features on SBUF partitions, tokens on the free dim
  - F=512-token chunks; flat software pipeline over (step t, chunk c)
  - layer-1 processing lags layer-0 by exactly one step (NCH slots), so both
    layers' states pair in the SAME state column: S[:, c] = [s0(c); s1(c)],
    where s0 is at step t and s1 at step t-1
  - every matmul is a K=128 full-row matmul with zero-padded weights,
    merged across layers (one Z matmul computes both z0 and z1); this
    avoids cross-row-tile PSUM accumulation, which faults on TRN2
  - elementwise cell update as 4 paired [128, F] DVE scalar_tensor_tensor
    ops with per-partition biases fused; Z/H PSUM banks evacuated to bf16
    SBUF by the scalar engine (ACT) to keep the DVE in 2x 16-bit mode
  - out-projections pair two iterations into one PSUM bank (weight in
    column 0 for even slots, column 64 for odd) -> one ACT evac + DMA
    per two iterations

Cell algebra per layer (no nonlinearities in this model):
  zp = s@Bz + x@Az ; rp likewise ; rs = (rp + br) * s
  hp = rs@Bh + x@Ah ; M = (hp + bh) - s ; P2 = (zp + bz) * M
  s' = (hp + bh) - P2   ==  z*s + (1-z)*h  with z = zp+bz, h = hp+bh
"""

import numpy as np
import ml_dtypes

import concourse.bass as bass
import concourse.tile as tile
from concourse import bacc, mybir
from concourse.bass_utils import run_bass_kernel_spmd

BF16 = np.float16
F = 512
HD = 64
NCORE = 8
OS = 2    # out-projection emission shift (iterations)


def _build(T, NCH, reps=1):
    NITER = T * NCH
    TOK = NCH * F
    dt = mybir.dt
    AO = mybir.AluOpType
    NSLAB = 6 + 3 * T
    WCOLS = NSLAB * 128
    CC = WCOLS + TOK  # constants blob columns: weight slabs then x columns

    nc = bacc.Bacc("TRN2", target_bir_lowering=False, debug=False,
                   num_devices=NCORE)
    cst_d = nc.dram_tensor("cst", [128, CC], dt.float16, kind="ExternalInput")
    sp_d = nc.dram_tensor("sp", [128, TOK], dt.float16, kind="ExternalInput")
    bp_d = nc.dram_tensor("bp", [128, 3], dt.float32, kind="ExternalInput")
    out_d = nc.dram_tensor("out", [T, TOK], dt.float32, kind="ExternalOutput")

    with tile.TileContext(nc) as tc:
        with (
            tc.tile_pool(name="const", bufs=1) as cpool,
            tc.tile_pool(name="state", bufs=1) as spool,
            tc.tile_pool(name="work", bufs=4) as wpool,
            tc.tile_pool(name="ostage", bufs=3) as opool,
            tc.tile_pool(name="psum", bufs=2, space="PSUM") as ppool,
        ):
            csb = cpool.tile([128, CC], dt.float16)
            bsb = cpool.tile([128, 3], dt.float32)
            S_all = spool.tile([128, TOK], dt.float16)
            # Load state columns individually so the first pipeline bodies
            # can start as soon as their own chunk has landed (bacc's
            # sync-wait legalization handles the fan-in; no barrier needed).
            nc.gpsimd.dma_start(out=csb[:], in_=cst_d[:])
            nc.gpsimd.dma_start(out=bsb[:], in_=bp_d[:])
            nc.gpsimd.dma_start(out=S_all[:], in_=sp_d[:])

            def slab(idx):
                return csb[:, idx * 128:(idx + 1) * 128]

            WZ, WR, WH, WHT, WOE, WOO = (slab(i) for i in range(6))
            bz, br, bh = (bsb[:, i:i + 1] for i in range(3))
            out_view = out_d[:].rearrange("t (c f) -> (t c) f", f=F)

            def Sfc(c):
                return S_all[:, c * F:(c + 1) * F]

            mm = nc.tensor.matmul
            stt = nc.vector.scalar_tensor_tensor
            tt = nc.vector.tensor_tensor
            act = nc.scalar.activation
            IDENT = mybir.ActivationFunctionType.Identity

            # Software pipeline: slot j's phases are emitted across bodies so
            # no engine's in-order queue ever waits on same-body work:
            #   body j:   Z/R matmuls + RS op (slot j)
            #   body j+1: H matmuls + Z/H evacs (slot j)
            #   body j+2: M/P2/S' update cluster (slot j)
            #   body j+3: out-projection (slot j)
            def active(j):
                return (j < NITER, NCH <= j < NITER + NCH)

            def emit_rep():
              zr_ph = {}
              up_ph = {}
              ps_o_box = [None]
              for i in range(NITER + NCH + 4):
                  l0, l1 = active(i)
                  if l0 or l1:
                      c = i % NCH
                      t = i // NCH
                      ps_z = ppool.tile([128, F], dt.float32, tag="Z")
                      ps_r = ppool.tile([128, F], dt.float32, tag="R")
                      S = Sfc(c)
                      if l0:
                          xap = csb[:, WCOLS + c * F:WCOLS + (c + 1) * F]
                          XZ, XR = (slab(6 + 3 * t + k) for k in range(2))
                      mm(ps_z[:], WZ, S, start=True, stop=not l0,
                         tile_position=(0, 0))
                      if l0:
                          mm(ps_z[:], XZ, xap, start=False, stop=True,
                             tile_position=(0, 0))
                      mm(ps_r[:], WR, S, start=True, stop=not l0,
                         tile_position=(0, 0))
                      if l0:
                          mm(ps_r[:], XR, xap, start=False, stop=True,
                             tile_position=(0, 0))
                      zr_ph[i] = (l0, l1, c, t, ps_z, ps_r)

                  # state-update cluster for slot i-2 (before RS so the DVE
                  # queue always has ready work first)
                  j = i - 2
                  if j in up_ph:
                      l0j, l1j, cj, Ztj, Htj = up_ph.pop(j)
                      Sj = Sfc(cj)
                      slj = (slice(0, 128) if (l0j and l1j)
                             else (slice(0, 64) if l0j else slice(64, 128)))
                      Mt = wpool.tile([128, F], dt.float16, tag="Mt")
                      P2 = wpool.tile([128, F], dt.float16, tag="P2")
                      # Ht/Zt already carry the +bh/+bz from the ACT evac, so
                      # the whole update cluster runs as fp16 SBUF
                      # tensor_tensor ops (2x DVE mode).
                      tt(out=Mt[:], in0=Htj[:], in1=Sj, op=AO.subtract)
                      tt(out=P2[:], in0=Ztj[:], in1=Mt[:], op=AO.mult)
                      tt(out=S_all[slj, cj * F:(cj + 1) * F],
                         in0=Htj[slj, :], in1=P2[slj, :], op=AO.subtract)

                  # RS op for slot i
                  if l0 or l1:
                      RS = wpool.tile([128, F], dt.float16, tag="RS")
                      stt(out=RS[:], in0=ps_r[:], scalar=br, in1=S,
                          op0=AO.add, op1=AO.mult)
                      zr_ph[i] = zr_ph[i] + (RS,)

                  # H matmuls + evacs for slot i-1
                  j = i - 1
                  if j in zr_ph:
                      l0j, l1j, cj, tj, ps_zj, ps_rj, RSj = zr_ph.pop(j)
                      Sj = Sfc(cj)
                      ps_h = ppool.tile([128, F], dt.float32, tag="H")
                      mm(ps_h[:], WH, RSj[:], start=True, stop=False,
                         tile_position=(0, 0))
                      if l0j:
                          XH = slab(6 + 3 * tj + 2)
                          xapj = csb[:, WCOLS + cj * F:WCOLS + (cj + 1) * F]
                          mm(ps_h[:], XH, xapj, start=False, stop=False,
                             tile_position=(0, 0))
                      mm(ps_h[:], WHT, Sj, start=False, stop=True,
                         tile_position=(0, 0))
                      Zt = wpool.tile([128, F], dt.float16, tag="Zt")
                      Ht = wpool.tile([128, F], dt.float16, tag="Ht")
                      act(out=Zt[:], in_=ps_zj[:], func=IDENT, bias=bz,
                          scale=1.0)
                      act(out=Ht[:], in_=ps_h[:], func=IDENT, bias=bh,
                          scale=1.0)
                      up_ph[j] = (l0j, l1j, cj, Zt, Ht)

                  # out-projection for slot oi = i - 3 (reads updated s1')
                  oi = i - 3
                  if NCH <= oi < NITER + NCH:
                      cs_out = oi - NCH
                      So = Sfc(oi % NCH)
                      if cs_out % 2 == 0:
                          ps_o = ppool.tile([128, F], dt.float32, tag="O")
                          ps_o_box[0] = ps_o
                          mm(ps_o[:], WOE, So, start=True, stop=False,
                             tile_position=(0, 0))
                      else:
                          ps_o = ps_o_box[0]
                          mm(ps_o[:], WOO, So, start=False, stop=True,
                             tile_position=(0, 0))
                          ost = opool.tile([128, F], dt.float32, tag="ost")
                          nc.scalar.copy(out=ost[:], in_=ps_o[:])
                          nc.gpsimd.dma_start(out=out_view[cs_out - 1:cs_out, :],
                                              in_=ost[0:1, :])
                          nc.gpsimd.dma_start(out=out_view[cs_out:cs_out + 1, :],
                                              in_=ost[64:65, :])

            if reps == 1:
                emit_rep()
            else:
                # Device-side loop: NEFF size is independent of `reps`, so a
                # reps=R run re-executes the same instruction stream R times
                # on-device (state keeps evolving; only used for timing).
                with tc.For_i(0, reps):
                    emit_rep()
    nc.compile()
    return nc


def _pack_weights(T, Wzr0, Wh0, Wzr1, Wh1, W_out):
    NSLAB = 6 + 3 * T
    wp = np.zeros((128, NSLAB * 128), np.float32)

    def sl(i):
        return wp[:, i * 128:(i + 1) * 128]

    # WZ: cols 0:64 -> z0 state part [Bz0; 0], cols 64:128 -> z1 = Wzr1 z-cols
    sl(0)[0:64, 0:64] = Wzr0[2:66, 0:HD]
    sl(0)[:, 64:128] = Wzr1[:, 0:HD]
    # WR
    sl(1)[0:64, 0:64] = Wzr0[2:66, HD:]
    sl(1)[:, 64:128] = Wzr1[:, HD:]
    # WH (rhs = RS = [rs0; rs1]): cols 0:64 -> [Bh0; 0], cols 64:128 -> [0; Wh1b]
    sl(2)[0:64, 0:64] = Wh0[2:66, :]
    sl(2)[64:128, 64:128] = Wh1[HD:, :]
    # WHT (rhs = S = [s0'; s1]): cols 64:128 -> [Wh1t; 0]
    sl(3)[0:64, 64:128] = Wh1[0:HD, :]
    # WOE / WOO: out projection, even slot in col 0, odd slot in col 64
    sl(4)[64:128, 0:1] = W_out
    sl(5)[64:128, 64:65] = W_out
    # x-projection slabs per step: rhs = x-region (x at rows 64+2t+d)
    for tt in range(T):
        for k, W in enumerate((Wzr0[0:2, 0:HD], Wzr0[0:2, HD:], Wh0[0:2, :])):
            s = sl(6 + 3 * tt + k)
            s[64 + 2 * tt:64 + 2 * tt + 2, 0:64] = W
    return wp


def _pack_biases(bzr0, bh0, bzr1, bh1):
    bp = np.zeros((128, 3), np.float32)
    bp[:, 0] = np.concatenate([bzr0[:HD], bzr1[:HD]])   # bz
    bp[:, 1] = np.concatenate([bzr0[HD:], bzr1[HD:]])   # br
    bp[:, 2] = np.concatenate([bh0, bh1])               # bh
    return bp


_NC_CACHE = {}


def _get_nc(T, NCH, reps=1):
    key = (T, NCH, reps)
    if key not in _NC_CACHE:
        _NC_CACHE[key] = _build(T, NCH, reps)
    return _NC_CACHE[key]


def run_cores(x_cores, s0_cores, s1_cores, T, NCH,
              Wzr0, bzr0, Wh0, bh0, Wzr1, bzr1, Wh1, bh1, W_out,
              **run_kwargs):
    """x_cores: list of [T, 2, TOK] fp32; s*_cores: list of [64, TOK] fp32.
    Returns (list of [T, TOK] fp32 outputs (without b_out), BassKernelResults)."""
    TOK = NCH * F
    nc = _get_nc(T, NCH, run_kwargs.pop("reps", 1))
    wp = _pack_weights(T, Wzr0, Wh0, Wzr1, Wh1, W_out)
    bp = _pack_biases(bzr0, bh0, bzr1, bh1)
    in_maps = []
    for x_core, s0, s1 in zip(x_cores, s0_cores, s1_cores):
        xpart = np.zeros((128, TOK), np.float32)
        xpart[64:64 + 2 * T] = x_core.reshape(2 * T, TOK)
        cst = np.concatenate([wp, xpart], axis=1).astype(BF16)
        sp = np.concatenate([s0, s1], 0).astype(BF16)
        in_maps.append({"cst": cst, "sp": sp, "bp": bp})
    res = run_bass_kernel_spmd(nc, in_maps, core_ids=list(range(len(in_maps))),
                               **run_kwargs)
    return [r["out"] for r in res.results], res


def kernel(x, init_state, Wzr0, bzr0, Wh0, bh0, Wzr1, bzr1, Wh1, bh1,
           W_out, b_out):
    x = np.asarray(x)
    init_state = np.asarray(init_state)
    args = [np.asarray(a) for a in
            (Wzr0, bzr0, Wh0, bh0, Wzr1, bzr1, Wh1, bh1, W_out)]
    b_out = np.asarray(b_out)

    B, T, N, Din = x.shape
    assert (B, T, N, Din) == (64, 12, 2048, 2), x.shape
    BPC = B // NCORE
    TOK = BPC * N
    NCH = TOK // F

    x_cores, s0_cores, s1_cores = [], [], []
    for core in range(NCORE):
        bs = slice(core * BPC, (core + 1) * BPC)
        x_cores.append(x[bs].transpose(1, 3, 0, 2).reshape(T, Din, TOK))
        s0_cores.append(init_state[0, bs].transpose(2, 0, 1).reshape(HD, TOK))
        s1_cores.append(init_state[1, bs].transpose(2, 0, 1).reshape(HD, TOK))

    outs, _ = run_cores(x_cores, s0_cores, s1_cores, T, NCH, *args)
    out = np.stack(outs).reshape(NCORE, T, BPC, N).transpose(0, 2, 1, 3)
    out = out.reshape(B, T, N, 1) + b_out[0]
    return out.astype(np.float32)

